# revision 3
# baseline (speedup 1.0000x reference)
"""Trainium2 Bass kernel for nn_Attention_55894704390617.

Dense transformer attention block:
  xn = LN(x) ; q,k,v = xn @ wq/wk/wv ; q,k = headLN(q),headLN(k)
  out = softmax(q k^T / sqrt(dh)) v @ wo

Sharding over 8 NeuronCores: 2 (batch) x 4 (head groups of 8 heads).
Each core computes a partial output (its head-group's contribution to
out = attn_out @ wo); the host sums the 4 partials per batch.

Per-core kernel data flow (all matmuls bf16, fp32 PSUM accumulation):
  - host supplies x[b] both natural [S,D] (for LN stats) and transposed
    [D,S] (contraction operand for the projections)
  - LN stats (mean/rstd per token) via bn_stats on natural tiles; the
    mean-correction of the projections is folded into each matmul as a
    K=1 accumulation row (mu[s] x -colsum(w)); rstd is applied
    per-partition on the PSUM results
  - Q,K: head-LN via bn_stats in natural layout, then PE-transposed to
    [m, s] for the attention matmuls
  - scores computed transposed (scoresT[j,i]) so softmax denominators
    come from a ones-column in v_ext via the attention matmul itself;
    no max-subtraction (scores are O(5), exp is fp32-safe)
  - out-proj consumes avT directly as the stationary operand
"""

import numpy as np

S = 2048          # sequence length
D = 2048          # model dim
H_LOC = 8         # heads per core
DH = 64           # head dim
M_LOC = H_LOC * DH  # 512 inner dim per core
N_D = D // 128    # 16 d-tiles
N_S = S // 128    # 16 s-tiles
N_SC = S // 512   # 4 512-chunks
N_MT = M_LOC // 128  # 4 m-tiles per core
EPS = 1e-5

_COMPILED = {}


def _build():
    from concourse._compat import axon_active
    axon_active()
    import concourse.bacc as bacc
    import concourse.mybir as mybir
    import concourse.tile as tile
    from concourse.bass import AP
    from concourse.masks import make_identity
    from contextlib import ExitStack

    F32 = mybir.dt.float32
    BF16 = mybir.dt.bfloat16
    AF = mybir.ActivationFunctionType
    OP = mybir.AluOpType

    nc = bacc.Bacc(None, target_bir_lowering=False)

    x_nat = nc.dram_tensor("x_nat", [S, D], F32, kind="ExternalInput")
    x_tr = nc.dram_tensor("x_tr", [D, S], F32, kind="ExternalInput")
    wq = nc.dram_tensor("wq", [D, M_LOC], F32, kind="ExternalInput")
    wk = nc.dram_tensor("wk", [D, M_LOC], F32, kind="ExternalInput")
    wv = nc.dram_tensor("wv", [D, M_LOC], F32, kind="ExternalInput")
    wo = nc.dram_tensor("wo", [M_LOC, D], F32, kind="ExternalInput")
    norm_w = nc.dram_tensor("norm_w", [D], F32, kind="ExternalInput")
    qn_w = nc.dram_tensor("qn_w", [DH], F32, kind="ExternalInput")
    kn_w = nc.dram_tensor("kn_w", [DH], F32, kind="ExternalInput")
    out = nc.dram_tensor("out", [S, D], F32, kind="ExternalOutput")

    with tile.TileContext(nc) as tc:
        es = ExitStack()
        # ---- pools alive for the whole kernel ----
        consts = es.enter_context(tc.tile_pool(name="consts", bufs=1))
        dram = es.enter_context(tc.tile_pool(name="dram", bufs=1, space="DRAM"))
        qkT_pool = es.enter_context(tc.tile_pool(name="qkT", bufs=1))
        vext_pool = es.enter_context(tc.tile_pool(name="vext", bufs=1))

        ident = consts.tile([128, 128], BF16, name="ident")
        make_identity(nc, ident)
        ones_all = consts.tile([128, 64], BF16, name="ones_all")
        nc.vector.memset(ones_all, 1.0)
        ones_col = consts.tile([128, 1], BF16, name="ones_col")
        nc.vector.memset(ones_col, 1.0)
        eps_t = consts.tile([128, 1], F32, name="eps_t")
        nc.vector.memset(eps_t, EPS)

        # per-partition replicas of qn_w/kn_w along the free (head) axis
        qn_rep = consts.tile([128, H_LOC, DH], F32, name="qn_rep")
        kn_rep = consts.tile([128, H_LOC, DH], F32, name="kn_rep")
        for rep, wten in ((qn_rep, qn_w), (kn_rep, kn_w)):
            src = AP(tensor=wten[:].tensor, offset=wten[:].offset,
                     ap=[[0, 128], [0, H_LOC], [1, DH]])
            nc.sync.dma_start(out=rep, in_=src)

        # norm_w as 16 per-partition column slices
        g_col = consts.tile([128, N_D], F32, name="g_col")
        for t in range(N_D):
            nc.sync.dma_start(
                out=g_col[:, t:t + 1],
                in_=norm_w[t * 128:(t + 1) * 128].rearrange("(p one) -> p one", one=1))

        mu_cols = consts.tile([128, N_S], F32, name="mu_cols")
        rstd_cols = consts.tile([128, N_S], F32, name="rstd_cols")
        mu_bf = consts.tile([1, S], BF16, name="mu_bf")
        negc_bf = {w: consts.tile([1, M_LOC], BF16, name=f"negc_{w}")
                   for w in ("q", "k", "v")}

        dscr = dram.tile([S], F32)

        qT = [qkT_pool.tile([128, S], BF16, name=f"qT{mt}") for mt in range(N_MT)]
        kT = [qkT_pool.tile([128, S], BF16, name=f"kT{mt}") for mt in range(N_MT)]
        v_ext = [vext_pool.tile([128, H_LOC, DH + 1], BF16, name=f"vext{st}")
                 for st in range(N_S)]

        # ================= phase A: prep + stats + projections =================
        with ExitStack() as ph:
            xt_pool = ph.enter_context(tc.tile_pool(name="xt", bufs=1))
            wg_pool = ph.enter_context(tc.tile_pool(name="wg", bufs=1))
            stage = ph.enter_context(tc.tile_pool(name="stage", bufs=2))
            wstage = ph.enter_context(tc.tile_pool(name="wstage", bufs=2))
            scr = ph.enter_context(tc.tile_pool(name="scr", bufs=2))
            ps_mm = ph.enter_context(tc.tile_pool(name="ps_mm", bufs=3, space="PSUM"))
            ps_tp = ph.enter_context(tc.tile_pool(name="ps_tp", bufs=2, space="PSUM"))
            ps_row = ph.enter_context(tc.tile_pool(name="ps_row", bufs=2, space="PSUM"))

            # -- fold norm_w into the projection weights, cast bf16, colsums --
            wg = {}
            for wname, wdram in (("q", wq), ("k", wk), ("v", wv)):
                wg[wname] = [wg_pool.tile([128, M_LOC], BF16, name=f"wg_{wname}{t}")
                             for t in range(N_D)]
                for t in range(N_D):
                    wst = wstage.tile([128, M_LOC], F32, tag="wst")
                    nc.sync.dma_start(out=wst, in_=wdram[t * 128:(t + 1) * 128, :])
                    nc.vector.tensor_scalar_mul(
                        out=wg[wname][t], in0=wst, scalar1=g_col[:, t:t + 1])
                cw_ps = ps_row.tile([1, M_LOC], F32, tag="cw")
                for t in range(N_D):
                    nc.tensor.matmul(cw_ps[:, :], ones_col[:, :], wg[wname][t][:, :],
                                     start=(t == 0), stop=(t == N_D - 1))
                nc.vector.tensor_scalar_mul(out=negc_bf[wname], in0=cw_ps, scalar1=-1.0)

            # -- x transposed: load + cast --
            xt = [xt_pool.tile([128, S], BF16, name=f"xt{t}") for t in range(N_D)]
            for t in range(N_D):
                xst = stage.tile([128, S], F32, tag="xst")
                nc.sync.dma_start(out=xst, in_=x_tr[t * 128:(t + 1) * 128, :])
                nc.vector.tensor_copy(xt[t], xst)

            # -- LN stats from natural tiles --
            for st in range(N_S):
                xst = stage.tile([128, S], F32, tag="xst")
                nc.sync.dma_start(out=xst, in_=x_nat[st * 128:(st + 1) * 128, :])
                xg = xst.rearrange("p (n f) -> p n f", f=512)
                bn = scr.tile([128, 4, 6], F32, tag="bn")
                for sg in range(4):
                    nc.vector.bn_stats(out=bn[:, sg, :], in_=xg[:, sg, :])
                mv = scr.tile([128, 2], F32, tag="mv")
                nc.vector.bn_aggr(out=mv, in_=bn)
                nc.vector.tensor_copy(mu_cols[:, st:st + 1], mv[:, 0:1])
                nc.scalar.activation(out=rstd_cols[:, st:st + 1], in_=mv[:, 1:2],
                                     func=AF.Sqrt, bias=eps_t, scale=1.0)
                nc.vector.reciprocal(rstd_cols[:, st:st + 1], rstd_cols[:, st:st + 1])

            # mean row form via DRAM round trip
            for t in range(N_S):
                nc.sync.dma_start(
                    out=dscr[t * 128:(t + 1) * 128].rearrange("(p one) -> p one", one=1),
                    in_=mu_cols[:, t:t + 1])
            mu_row = consts.tile([1, S], F32, name="mu_row")
            nc.sync.dma_start(out=mu_row,
                              in_=dscr[:].rearrange("(one s) -> one s", one=1))
            nc.vector.tensor_copy(mu_bf, mu_row)

            # -- projections --
            for st in range(N_S):
                sl = slice(st * 128, (st + 1) * 128)
                for wname in ("q", "k", "v"):
                    p = ps_mm.tile([128, M_LOC], F32, tag="mm")
                    for t in range(N_D):
                        nc.tensor.matmul(p[:, :], xt[t][:, sl], wg[wname][t][:, :],
                                         start=(t == 0), stop=False)
                    nc.tensor.matmul(p[:, :], mu_bf[:, sl], negc_bf[wname][:, :],
                                     start=False, stop=True)
                    if wname == "v":
                        nc.vector.tensor_scalar_mul(
                            out=v_ext[st][:, :, 0:DH],
                            in0=p.rearrange("p (h d) -> p h d", d=DH),
                            scalar1=rstd_cols[:, st:st + 1])
                        nc.vector.memset(v_ext[st][:, :, DH:DH + 1], 1.0)
                    else:
                        nat = scr.tile([128, M_LOC], F32, tag="nat")
                        nc.vector.tensor_scalar_mul(
                            out=nat, in0=p, scalar1=rstd_cols[:, st:st + 1])
                        natg = nat.rearrange("p (h d) -> p h d", d=DH)
                        bn8 = scr.tile([128, H_LOC, 6], F32, tag="bn8")
                        mv8 = scr.tile([128, H_LOC, 2], F32, tag="mv8")
                        for h in range(H_LOC):
                            nc.vector.bn_stats(out=bn8[:, h, :], in_=natg[:, h, :])
                            nc.vector.bn_aggr(out=mv8[:, h, :], in_=bn8[:, h, :])
                        rstd8 = scr.tile([128, H_LOC], F32, tag="rstd8")
                        nc.scalar.activation(out=rstd8, in_=mv8[:, :, 1], func=AF.Sqrt,
                                             bias=eps_t, scale=1.0)
                        nc.vector.reciprocal(rstd8, rstd8)
                        for h in range(H_LOC):
                            nc.vector.tensor_scalar(
                                out=natg[:, h, :], in0=natg[:, h, :],
                                scalar1=mv8[:, h, 0:1], scalar2=rstd8[:, h:h + 1],
                                op0=OP.subtract, op1=OP.mult)
                        lnb = scr.tile([128, M_LOC], BF16, tag="lnb")
                        rep = qn_rep if wname == "q" else kn_rep
                        nc.vector.tensor_mul(
                            out=lnb, in0=nat,
                            in1=rep.rearrange("p h d -> p (h d)"))
                        dst = qT if wname == "q" else kT
                        for mt in range(N_MT):
                            tp = ps_tp.tile([128, 128], BF16, tag="tp")
                            nc.tensor.transpose(
                                tp[:, :], lnb[:, mt * 128:(mt + 1) * 128], ident[:, :])
                            nc.vector.tensor_copy(dst[mt][:, sl], tp[:, :])

        # ================= phase B: attention + out-proj =================
        with ExitStack() as ph:
            wo_pool = ph.enter_context(tc.tile_pool(name="wop", bufs=1))
            stage2 = ph.enter_context(tc.tile_pool(name="stage2", bufs=2))
            avT_pool = ph.enter_context(tc.tile_pool(name="avT", bufs=1))
            attn_pool = ph.enter_context(tc.tile_pool(name="attn", bufs=6))
            dn_pool = ph.enter_context(tc.tile_pool(name="dn", bufs=2))
            avsh_pool = ph.enter_context(tc.tile_pool(name="avsh", bufs=2))
            osb_pool = ph.enter_context(tc.tile_pool(name="osb", bufs=3))
            ps_sc = ph.enter_context(tc.tile_pool(name="ps_sc", bufs=2, space="PSUM"))
            ps_av = ph.enter_context(tc.tile_pool(name="ps_av", bufs=2, space="PSUM"))
            ps_bc = ph.enter_context(tc.tile_pool(name="ps_bc", bufs=1, space="PSUM"))
            ps_op = ph.enter_context(tc.tile_pool(name="ps_op", bufs=2, space="PSUM"))

            wo_bf = [wo_pool.tile([128, D], BF16, name=f"wo{mt}") for mt in range(N_MT)]
            for mt in range(N_MT):
                wst = stage2.tile([128, D], F32, tag="wst2")
                nc.sync.dma_start(out=wst, in_=wo[mt * 128:(mt + 1) * 128, :])
                nc.vector.tensor_copy(wo_bf[mt], wst)

            avT = [avT_pool.tile([128, S], BF16, name=f"avT{mt}") for mt in range(N_MT)]

            for ic in range(N_SC):
                isl = slice(ic * 512, (ic + 1) * 512)
                for hp in range(N_MT):
                    av_ps = [ps_av.tile([128, 512], F32, tag="av", name=f"av{_h}")
                             for _h in range(2)]
                    for jt in range(N_S):
                        jsl = slice(jt * 128, (jt + 1) * 128)
                        at = []
                        for hs in range(2):
                            psl = slice(hs * 64, (hs + 1) * 64)
                            sc = ps_sc.tile([128, 512], F32, tag="sc")
                            nc.tensor.matmul(sc[:, :], kT[hp][psl, jsl],
                                             qT[hp][psl, isl], start=True, stop=True)
                            a = attn_pool.tile([128, 512], BF16, tag="attn")
                            nc.scalar.activation(out=a, in_=sc, func=AF.Exp, scale=0.125)
                            at.append(a)
                        for hs in range(2):
                            nc.tensor.matmul(av_ps[hs][0:DH + 1, :],
                                             v_ext[jt][:, 2 * hp + hs, :], at[hs][:, :],
                                             start=(jt == 0), stop=(jt == N_S - 1))
                    for hs in range(2):
                        nc.vector.reciprocal(av_ps[hs][DH:DH + 1, :],
                                             av_ps[hs][DH:DH + 1, :])
                        dn = dn_pool.tile([65, 512], BF16, tag="dn")
                        nc.vector.tensor_copy(dn[DH:DH + 1, :], av_ps[hs][DH:DH + 1, :])
                        bc = ps_bc.tile([64, 512], F32, tag="bc")
                        nc.tensor.matmul(bc[:, :], ones_all[DH:DH + 1, 0:DH],
                                         dn[DH:DH + 1, :], start=True, stop=True)
                        bc_sb = dn_pool.tile([64, 512], F32, tag="bcsb")
                        nc.vector.tensor_copy(bc_sb, bc[:, :])
                        if hs == 0:
                            nc.vector.tensor_mul(out=avT[hp][0:DH, isl],
                                                 in0=av_ps[hs][0:DH, :], in1=bc_sb)
                        else:
                            sh = avsh_pool.tile([64, 512], BF16, tag="avsh")
                            nc.vector.tensor_mul(out=sh, in0=av_ps[hs][0:DH, :],
                                                 in1=bc_sb)
                            nc.sync.dma_start(out=avT[hp][DH:128, isl], in_=sh)
                # out-projection for the four s-tiles of this chunk
                for st in range(4 * ic, 4 * ic + 4):
                    sl = slice(st * 128, (st + 1) * 128)
                    for do in range(N_SC):
                        op = ps_op.tile([128, 512], F32, tag="op")
                        for mt in range(N_MT):
                            nc.tensor.matmul(op[:, :], avT[mt][:, sl],
                                             wo_bf[mt][:, do * 512:(do + 1) * 512],
                                             start=(mt == 0), stop=(mt == N_MT - 1))
                        ot = osb_pool.tile([128, 512], F32, tag="ot")
                        nc.vector.tensor_copy(ot, op)
                        nc.sync.dma_start(out=out[sl, do * 512:(do + 1) * 512], in_=ot)
        es.close()

    nc.compile()
    return nc


def _get_nc():
    if "nc" not in _COMPILED:
        _COMPILED["nc"] = _build()
    return _COMPILED["nc"]


def kernel(x, norm_w, wq, wk, wv, qn_w, kn_w, wo):
    from concourse.bass_utils import run_bass_kernel_spmd

    x = np.asarray(x, dtype=np.float32)
    norm_w = np.asarray(norm_w, dtype=np.float32)
    wq = np.asarray(wq, dtype=np.float32)
    wk = np.asarray(wk, dtype=np.float32)
    wv = np.asarray(wv, dtype=np.float32)
    qn_w = np.asarray(qn_w, dtype=np.float32)
    kn_w = np.asarray(kn_w, dtype=np.float32)
    wo = np.asarray(wo, dtype=np.float32)
    B = x.shape[0]

    nc = _get_nc()
    in_maps = []
    for c in range(8):
        b, g = c // 4, c % 4
        ms = slice(g * M_LOC, (g + 1) * M_LOC)
        in_maps.append({
            "x_nat": np.ascontiguousarray(x[b]),
            "x_tr": np.ascontiguousarray(x[b].T),
            "wq": np.ascontiguousarray(wq[:, ms]),
            "wk": np.ascontiguousarray(wk[:, ms]),
            "wv": np.ascontiguousarray(wv[:, ms]),
            "wo": np.ascontiguousarray(wo[ms, :]),
            "norm_w": norm_w,
            "qn_w": qn_w,
            "kn_w": kn_w,
        })
    res = run_bass_kernel_spmd(nc, in_maps, core_ids=list(range(8)))
    out = np.zeros((B, S, D), dtype=np.float32)
    for c in range(8):
        out[c // 4] += res.results[c]["out"]
    return out


# revision 14
# speedup vs baseline: 91.0167x; 91.0167x over previous
"""Trainium2 Bass kernel for nn_Attention_55894704390617.

Dense transformer attention block:
  xn = LN(x) ; q,k,v = xn @ wq/wk/wv ; q,k = headLN(q),headLN(k)
  out = softmax(q k^T / sqrt(dh)) v @ wo

Sharding over 8 NeuronCores: 2 (batch) x 4 (head groups of 8 heads).
Each core computes a partial output (its head-group's contribution to
out = attn_out @ wo); the host sums the 4 partials per batch.

Per-core data flow (matmuls in bf16, fp32 PSUM accumulation):
  - host supplies x[b] natural [S,D] fp32 (LN stats) and transposed
    [D,S] pre-cast to bf16 (the projections' contraction operand)
  - LN stats per token via bn_stats on natural tiles; each projection's
    mean-correction is folded into its matmul group as a K=1
    accumulation row (mu[s] x -colsum(w)); rstd applied per-partition
    on the PSUM results; norm_w folded into the weights on ScalarE
  - K,V projected in phase A; Q projected per 512-query chunk inside
    the attention loop so its PE work fills softmax (ACT) stalls
  - scores computed transposed (scoresT[j,i]) with two heads packed
    into PE row-groups 0/64; softmax denominators come from a
    ones-column in v_ext through the attention matmul (no
    max-subtraction: scores are O(5), fp32-safe)
  - attn output normalized from an SBUF copy (frees the PSUM
    accumulator early); denominator broadcast via a K=1 ones-matmul
  - out-proj consumes avT directly as the stationary operand
"""

import numpy as np

S = 2048          # sequence length
D = 2048          # model dim
H_LOC = 8         # heads per core
DH = 64           # head dim
M_LOC = H_LOC * DH  # 512 inner dim per core
N_D = D // 128    # 16 d-tiles
N_S = S // 128    # 16 s-tiles
N_SC = S // 512   # 4 512-chunks
N_MT = M_LOC // 128  # 4 m-tiles per core
EPS = 1e-5

_COMPILED = {}


def _build():
    from concourse._compat import axon_active
    axon_active()
    import concourse.bacc as bacc
    import concourse.mybir as mybir
    import concourse.tile as tile
    from concourse.bass import AP
    from concourse.masks import make_identity
    from contextlib import ExitStack

    F32 = mybir.dt.float32
    BF16 = mybir.dt.bfloat16
    AF = mybir.ActivationFunctionType
    OP = mybir.AluOpType

    nc = bacc.Bacc(None, target_bir_lowering=False)

    x_nat = nc.dram_tensor("x_nat", [S, D], BF16, kind="ExternalInput")
    x_tr = nc.dram_tensor("x_tr", [D, S], BF16, kind="ExternalInput")
    wq = nc.dram_tensor("wq", [D, M_LOC], BF16, kind="ExternalInput")
    wk = nc.dram_tensor("wk", [D, M_LOC], BF16, kind="ExternalInput")
    wv = nc.dram_tensor("wv", [D, M_LOC], BF16, kind="ExternalInput")
    wo = nc.dram_tensor("wo", [M_LOC, D], BF16, kind="ExternalInput")
    norm_w = nc.dram_tensor("norm_w", [D], F32, kind="ExternalInput")
    qn_w = nc.dram_tensor("qn_w", [DH], F32, kind="ExternalInput")
    kn_w = nc.dram_tensor("kn_w", [DH], F32, kind="ExternalInput")
    out = nc.dram_tensor("out", [S, D], F32, kind="ExternalOutput")

    with tile.TileContext(nc) as tc:
        es = ExitStack()
        # ---- pools alive for the whole kernel ----
        consts = es.enter_context(tc.tile_pool(name="consts", bufs=1))
        dram = es.enter_context(tc.tile_pool(name="dram", bufs=1, space="DRAM"))
        xt_pool = es.enter_context(tc.tile_pool(name="xt", bufs=1))
        wgq_pool = es.enter_context(tc.tile_pool(name="wgq", bufs=1))
        kT_pool = es.enter_context(tc.tile_pool(name="kT", bufs=1))
        vext_pool = es.enter_context(tc.tile_pool(name="vext", bufs=1))

        ident = consts.tile([128, 128], BF16, name="ident")
        make_identity(nc, ident)
        ones_all = consts.tile([128, 64], BF16, name="ones_all")
        nc.vector.memset(ones_all, 1.0)
        ones_col = consts.tile([128, 1], BF16, name="ones_col")
        nc.vector.memset(ones_col, 1.0)
        eps_t = consts.tile([128, 1], F32, name="eps_t")
        nc.vector.memset(eps_t, EPS)

        qn_rep = consts.tile([128, H_LOC, DH], F32, name="qn_rep")
        kn_rep = consts.tile([128, H_LOC, DH], F32, name="kn_rep")
        g_col = consts.tile([128, N_D], F32, name="g_col")

        # per-s-tile stat tiles (separate tiles keep deps per-tile)
        mu_col = [consts.tile([128, 1], F32, name=f"mu_col{t}") for t in range(N_S)]
        rstd_col = [consts.tile([128, 1], F32, name=f"rstd_col{t}") for t in range(N_S)]
        mu_bf = [consts.tile([1, 128], BF16, name=f"mu_bf{t}") for t in range(N_S)]
        negc_bf = {w: consts.tile([1, M_LOC], BF16, name=f"negc_{w}")
                   for w in ("q", "k", "v")}

        dscr = dram.tile([S], F32)

        wg_q = [wgq_pool.tile([128, M_LOC], BF16, name=f"wg_q{t}") for t in range(N_D)]
        xt = [xt_pool.tile([128, S], BF16, name=f"xt{t}") for t in range(N_D)]
        kT = [kT_pool.tile([128, S], BF16, name=f"kT{mt}") for mt in range(N_MT)]
        v_ext = [vext_pool.tile([128, H_LOC, DH + 1], BF16, name=f"vext{st}")
                 for st in range(N_S)]

        # ============ phase A: prep + stats + K,V projections ============
        with ExitStack() as ph:
            wg_pool = ph.enter_context(tc.tile_pool(name="wg", bufs=1))
            stage = ph.enter_context(tc.tile_pool(name="stage", bufs=2))
            wstage = ph.enter_context(tc.tile_pool(name="wstage", bufs=3))
            scrA = ph.enter_context(tc.tile_pool(name="scrA", bufs=2))
            ps_mm = ph.enter_context(tc.tile_pool(name="ps_mm", bufs=3, space="PSUM"))
            ps_tp = ph.enter_context(tc.tile_pool(name="ps_tp", bufs=2, space="PSUM"))
            ps_row = ph.enter_context(tc.tile_pool(name="ps_row", bufs=2, space="PSUM"))

            wg = {"q": wg_q}
            for wname in ("k", "v"):
                wg[wname] = [wg_pool.tile([128, M_LOC], BF16, name=f"wg_{wname}{t}")
                             for t in range(N_D)]
            wdrams = {"q": wq, "k": wk, "v": wv}

            def emit_stats(st):
                xst = stage.tile([128, S], BF16, tag="xst")
                nc.sync.dma_start(out=xst, in_=x_nat[st * 128:(st + 1) * 128, :])
                xg = xst.rearrange("p (n f) -> p n f", f=512)
                bn = scrA.tile([128, 4, 6], F32, tag="bn")
                for sg in range(4):
                    nc.vector.bn_stats(out=bn[:, sg, :], in_=xg[:, sg, :])
                mv = scrA.tile([128, 2], F32, tag="mv")
                nc.vector.bn_aggr(out=mv, in_=bn)
                nc.vector.tensor_copy(mu_col[st], mv[:, 0:1])
                nc.scalar.activation(out=rstd_col[st], in_=mv[:, 1:2],
                                     func=AF.Sqrt, bias=eps_t, scale=1.0)
                nc.vector.reciprocal(rstd_col[st], rstd_col[st])
                nc.sync.dma_start(
                    out=dscr[st * 128:(st + 1) * 128].rearrange("(p one) -> p one", one=1),
                    in_=mu_col[st])
                mur = scrA.tile([1, 128], F32, tag="mur")
                nc.sync.dma_start(
                    out=mur,
                    in_=dscr[st * 128:(st + 1) * 128].rearrange("(one s) -> one s", one=1))
                nc.vector.tensor_copy(mu_bf[st], mur)

            # interleave K-weight folds with the x^T loads so the first
            # K-projection can start as early as the DMAs allow
            cwk_ps = ps_row.tile([1, M_LOC], F32, tag="cw", name="cw_k")
            for t in range(N_D):
                nc.sync.dma_start(
                    out=g_col[:, t:t + 1],
                    in_=norm_w[t * 128:(t + 1) * 128].rearrange("(p one) -> p one",
                                                               one=1))
                wst = wstage.tile([128, M_LOC], BF16, tag="wst")
                nc.sync.dma_start(out=wst, in_=wk[t * 128:(t + 1) * 128, :])
                nc.scalar.mul(out=wg["k"][t], in_=wst, mul=g_col[:, t:t + 1])
                nc.tensor.matmul(cwk_ps[:, :], ones_col[:, :], wg["k"][t][:, :],
                                 start=(t == 0), stop=(t == N_D - 1))
                nc.sync.dma_start(out=xt[t], in_=x_tr[t * 128:(t + 1) * 128, :])
                if t == 0:
                    emit_stats(0)
            nc.vector.tensor_scalar_mul(out=negc_bf["k"], in0=cwk_ps, scalar1=-1.0)

            for rep, wten in ((qn_rep, qn_w), (kn_rep, kn_w)):
                bsrc = AP(tensor=wten[:].tensor, offset=wten[:].offset,
                          ap=[[0, 128], [0, H_LOC], [1, DH]])
                nc.sync.dma_start(out=rep, in_=bsrc)

            def emit_folds(wname):
                # stage weight, fold norm_w on ScalarE, colsum on PE
                cw_ps = ps_row.tile([1, M_LOC], F32, tag="cw", name=f"cw_{wname}")
                for t in range(N_D):
                    wst = wstage.tile([128, M_LOC], BF16, tag="wst")
                    nc.sync.dma_start(out=wst,
                                      in_=wdrams[wname][t * 128:(t + 1) * 128, :])
                    nc.scalar.mul(out=wg[wname][t], in_=wst, mul=g_col[:, t:t + 1])
                    nc.tensor.matmul(cw_ps[:, :], ones_col[:, :], wg[wname][t][:, :],
                                     start=(t == 0), stop=(t == N_D - 1))
                nc.vector.tensor_scalar_mul(out=negc_bf[wname], in0=cw_ps,
                                            scalar1=-1.0)

            emit_folds("v")

            # LN stats for the remaining tiles
            for st in range(1, N_S):
                emit_stats(st)

            # K and V projections
            for st in range(N_S):
                sl = slice(st * 128, (st + 1) * 128)
                for wname in ("k", "v"):
                    p = ps_mm.tile([128, M_LOC], F32, tag="mm")
                    for t in range(N_D):
                        nc.tensor.matmul(p[:, :], xt[t][:, sl], wg[wname][t][:, :],
                                         start=(t == 0), stop=False)
                    nc.tensor.matmul(p[:, :], mu_bf[st][:, :], negc_bf[wname][:, :],
                                     start=False, stop=True)
                    if wname == "v":
                        nc.scalar.mul(
                            out=v_ext[st][:, :, 0:DH],
                            in_=p.rearrange("p (h d) -> p h d", d=DH),
                            mul=rstd_col[st])
                        nc.vector.memset(v_ext[st][:, :, DH:DH + 1], 1.0)
                    else:
                        nat = scrA.tile([128, M_LOC], F32, tag="nat")
                        nc.scalar.mul(out=nat, in_=p, mul=rstd_col[st])
                        natg = nat.rearrange("p (h d) -> p h d", d=DH)
                        bn8 = scrA.tile([128, H_LOC, 6], F32, tag="bn8")
                        mv8 = scrA.tile([128, H_LOC, 2], F32, tag="mv8")
                        for h in range(H_LOC):
                            nc.vector.bn_stats(out=bn8[:, h, :], in_=natg[:, h, :])
                            nc.vector.bn_aggr(out=mv8[:, h, :], in_=bn8[:, h, :])
                        rstd8 = scrA.tile([128, H_LOC], F32, tag="rstd8")
                        nc.scalar.activation(out=rstd8, in_=mv8[:, :, 1], func=AF.Sqrt,
                                             bias=eps_t, scale=1.0)
                        nc.vector.reciprocal(rstd8, rstd8)
                        for h in range(H_LOC):
                            nc.vector.tensor_scalar(
                                out=natg[:, h, :], in0=natg[:, h, :],
                                scalar1=mv8[:, h, 0:1], scalar2=rstd8[:, h:h + 1],
                                op0=OP.subtract, op1=OP.mult)
                        lnb = scrA.tile([128, M_LOC], BF16, tag="lnb")
                        nc.vector.tensor_mul(out=lnb, in0=nat,
                                             in1=kn_rep.rearrange("p h d -> p (h d)"))
                        for mt in range(N_MT):
                            tp = ps_tp.tile([128, 128], BF16, tag="tp")
                            nc.tensor.transpose(
                                tp[:, :], lnb[:, mt * 128:(mt + 1) * 128], ident[:, :])
                            nc.vector.tensor_copy(kT[mt][:, sl], tp[:, :])

            # Q weight fold last: its DMAs land while the projections run
            emit_folds("q")

        # ============ phase B: Q proj + attention + out-proj ============
        with ExitStack() as ph:
            wo_pool = ph.enter_context(tc.tile_pool(name="wop", bufs=1))
            stage2 = ph.enter_context(tc.tile_pool(name="stage2", bufs=2))
            avT_pool = ph.enter_context(tc.tile_pool(name="avT", bufs=2))
            qT_pool = ph.enter_context(tc.tile_pool(name="qT", bufs=2))
            scrB = ph.enter_context(tc.tile_pool(name="scrB", bufs=2))
            attn_pool = ph.enter_context(tc.tile_pool(name="attn", bufs=6))
            dn_pool = ph.enter_context(tc.tile_pool(name="dn", bufs=2))
            avsh_pool = ph.enter_context(tc.tile_pool(name="avsh", bufs=2))
            osb_pool = ph.enter_context(tc.tile_pool(name="osb", bufs=2))
            # PSUM: sc x2 + av x2 + qp x2 + tpb x2 = 8 banks. qp also takes
            # the out-proj accumulators; tpb also takes the denominator
            # broadcasts.
            ps_sc = ph.enter_context(tc.tile_pool(name="ps_sc", bufs=2, space="PSUM"))
            ps_av = ph.enter_context(tc.tile_pool(name="ps_av", bufs=2, space="PSUM"))
            ps_qp = ph.enter_context(tc.tile_pool(name="ps_qp", bufs=2, space="PSUM"))
            ps_tpb = ph.enter_context(tc.tile_pool(name="ps_tpb", bufs=2, space="PSUM"))

            wo_bf = [wo_pool.tile([128, D], BF16, name=f"wo{mt}") for mt in range(N_MT)]
            for mt in range(N_MT):
                wst = stage2.tile([128, D], BF16, tag="wst2")
                nc.sync.dma_start(out=wst, in_=wo[mt * 128:(mt + 1) * 128, :])
                nc.vector.tensor_copy(wo_bf[mt], wst)

            def alloc_qT(ic):
                return [qT_pool.tile([128, 512], BF16, tag=f"qT{mt}",
                                     name=f"qT{mt}_{ic}") for mt in range(N_MT)]

            def emit_qproj_tail(st, p, qT):
                # everything after the matmul accumulation for one Q s-tile
                ssl = slice((st % 4) * 128, (st % 4 + 1) * 128)
                nc.tensor.matmul(p[:, :], mu_bf[st][:, :], negc_bf["q"][:, :],
                                 start=False, stop=True)
                nat = scrB.tile([128, M_LOC], F32, tag="nat")
                nc.vector.tensor_scalar_mul(out=nat, in0=p, scalar1=rstd_col[st])
                natg = nat.rearrange("p (h d) -> p h d", d=DH)
                bn8 = scrB.tile([128, H_LOC, 6], F32, tag="bn8")
                mv8 = scrB.tile([128, H_LOC, 2], F32, tag="mv8")
                for h in range(H_LOC):
                    nc.vector.bn_stats(out=bn8[:, h, :], in_=natg[:, h, :])
                    nc.vector.bn_aggr(out=mv8[:, h, :], in_=bn8[:, h, :])
                rstd8 = scrB.tile([128, H_LOC], F32, tag="rstd8")
                nc.scalar.activation(out=rstd8, in_=mv8[:, :, 1], func=AF.Sqrt,
                                     bias=eps_t, scale=1.0)
                nc.vector.reciprocal(rstd8, rstd8)
                for h in range(H_LOC):
                    nc.vector.tensor_scalar(
                        out=natg[:, h, :], in0=natg[:, h, :],
                        scalar1=mv8[:, h, 0:1], scalar2=rstd8[:, h:h + 1],
                        op0=OP.subtract, op1=OP.mult)
                lnb = scrB.tile([128, M_LOC], BF16, tag="lnb")
                nc.vector.tensor_mul(out=lnb, in0=nat,
                                     in1=qn_rep.rearrange("p h d -> p (h d)"))
                for mt in range(N_MT):
                    tp = ps_tpb.tile([128, 128], BF16, tag="tpb",
                                     name=f"tp{st}_{mt}")
                    nc.tensor.transpose(
                        tp[:, :], lnb[:, mt * 128:(mt + 1) * 128], ident[:, :])
                    nc.vector.tensor_copy(qT[mt][:, ssl], tp[:, :])

            def emit_qproj_full(ic, qT):
                for st in range(4 * ic, 4 * ic + 4):
                    sl = slice(st * 128, (st + 1) * 128)
                    p = ps_qp.tile([128, M_LOC], F32, tag="qp", name=f"qp{st}")
                    for t in range(N_D):
                        nc.tensor.matmul(p[:, :], xt[t][:, sl], wg_q[t][:, :],
                                         start=(t == 0), stop=False)
                    emit_qproj_tail(st, p, qT)

            def emit_oproj_group(avT_prev, ic_prev, st, do):
                sl = slice(st * 128, (st + 1) * 128)
                lsl = slice((st % 4) * 128, (st % 4 + 1) * 128)
                op = ps_qp.tile([128, 512], F32, tag="qp", name=f"op{st}_{do}")
                for mt in range(N_MT):
                    nc.tensor.matmul(op[:, :], avT_prev[mt][:, lsl],
                                     wo_bf[mt][:, do * 512:(do + 1) * 512],
                                     start=(mt == 0), stop=(mt == N_MT - 1))
                ot = osb_pool.tile([128, 512], F32, tag="ot")
                nc.vector.tensor_copy(ot, op)
                nc.sync.dma_start(out=out[sl, do * 512:(do + 1) * 512], in_=ot)

            qT_cur = alloc_qT(0)
            emit_qproj_full(0, qT_cur)
            avT_prev = None
            for ic in range(N_SC):
                qT = qT_cur
                qT_next = alloc_qT(ic + 1) if ic + 1 < N_SC else None
                avT = [avT_pool.tile([128, 512], BF16, tag=f"avT{mt}",
                                     name=f"avT{mt}_{ic}") for mt in range(N_MT)]
                for hp in range(N_MT):
                    av_ps = [ps_av.tile([128, 512], F32, tag="av", name=f"av{_h}")
                             for _h in range(2)]
                    # this hp's interleaved extra PE work:
                    # - s-tile hp of the next chunk's Q projection
                    # - s-tile hp of the previous chunk's out-projection
                    qst = 4 * (ic + 1) + hp if qT_next is not None else None
                    qp = None
                    if qst is not None:
                        qp = ps_qp.tile([128, M_LOC], F32, tag="qp", name=f"qp{qst}")
                    for jt in range(N_S):
                        jsl = slice(jt * 128, (jt + 1) * 128)
                        at = []
                        for hs in range(2):
                            psl = slice(hs * 64, (hs + 1) * 64)
                            sc = ps_sc.tile([128, 512], F32, tag="sc",
                                            name=f"sc{jt}_{hs}")
                            nc.tensor.matmul(sc[:, :], kT[hp][psl, jsl],
                                             qT[hp][psl, :], start=True, stop=True)
                            a = attn_pool.tile([128, 512], BF16, tag="attn")
                            nc.scalar.activation(out=a, in_=sc, func=AF.Exp,
                                                 scale=0.125)
                            at.append(a)
                        for hs in range(2):
                            nc.tensor.matmul(av_ps[hs][0:DH + 1, :],
                                             v_ext[jt][:, 2 * hp + hs, :],
                                             at[hs][:, :],
                                             start=(jt == 0), stop=(jt == N_S - 1))
                        if qp is not None:
                            qsl = slice(qst * 128, (qst + 1) * 128)
                            nc.tensor.matmul(qp[:, :], xt[jt][:, qsl],
                                             wg_q[jt][:, :],
                                             start=(jt == 0), stop=False)
                        if avT_prev is not None and jt % 4 == 0:
                            emit_oproj_group(avT_prev, ic - 1,
                                             4 * (ic - 1) + hp, jt // 4)
                    if qp is not None:
                        emit_qproj_tail(qst, qp, qT_next)
                    for hs in range(2):
                        # free the PSUM accumulator with one copy, then
                        # normalize out of SBUF
                        av_sb = dn_pool.tile([65, 512], F32, tag="avsb")
                        nc.vector.tensor_copy(av_sb, av_ps[hs][0:DH + 1, :])
                        nc.vector.reciprocal(av_sb[DH:DH + 1, :], av_sb[DH:DH + 1, :])
                        dn = dn_pool.tile([65, 512], BF16, tag="dn")
                        nc.vector.tensor_copy(dn[DH:DH + 1, :], av_sb[DH:DH + 1, :])
                        bc = ps_tpb.tile([64, 512], F32, tag="tpb", name=f"bc{hs}")
                        nc.tensor.matmul(bc[:, :], ones_all[DH:DH + 1, 0:DH],
                                         dn[DH:DH + 1, :], start=True, stop=True)
                        bc_sb = dn_pool.tile([64, 512], F32, tag="bcsb")
                        nc.vector.tensor_copy(bc_sb, bc[:, :])
                        if hs == 0:
                            nc.vector.tensor_mul(out=avT[hp][0:DH, :],
                                                 in0=av_sb[0:DH, :], in1=bc_sb)
                        else:
                            sh = avsh_pool.tile([64, 512], BF16, tag="avsh")
                            nc.vector.tensor_mul(out=sh, in0=av_sb[0:DH, :],
                                                 in1=bc_sb)
                            nc.sync.dma_start(out=avT[hp][DH:128, :], in_=sh)
                avT_prev = avT
                qT_cur = qT_next

            # tail: out-projection of the last chunk
            for st in range(4 * (N_SC - 1), 4 * N_SC):
                for do in range(N_SC):
                    emit_oproj_group(avT_prev, N_SC - 1, st, do)
        es.close()

    nc.compile()
    return nc


def _get_nc():
    if "nc" not in _COMPILED:
        _COMPILED["nc"] = _build()
    return _COMPILED["nc"]


def kernel(x, norm_w, wq, wk, wv, qn_w, kn_w, wo):
    import ml_dtypes
    from concourse.bass_utils import run_bass_kernel_spmd

    x = np.asarray(x, dtype=np.float32)
    norm_w = np.asarray(norm_w, dtype=np.float32)
    wq = np.asarray(wq, dtype=np.float32)
    wk = np.asarray(wk, dtype=np.float32)
    wv = np.asarray(wv, dtype=np.float32)
    qn_w = np.asarray(qn_w, dtype=np.float32)
    kn_w = np.asarray(kn_w, dtype=np.float32)
    wo = np.asarray(wo, dtype=np.float32)
    B = x.shape[0]

    nc = _get_nc()
    in_maps = []
    for c in range(8):
        b, g = c // 4, c % 4
        ms = slice(g * M_LOC, (g + 1) * M_LOC)
        in_maps.append({
            "x_nat": np.ascontiguousarray(x[b]).astype(ml_dtypes.bfloat16),
            "x_tr": np.ascontiguousarray(x[b].T).astype(ml_dtypes.bfloat16),
            "wq": np.ascontiguousarray(wq[:, ms]).astype(ml_dtypes.bfloat16),
            "wk": np.ascontiguousarray(wk[:, ms]).astype(ml_dtypes.bfloat16),
            "wv": np.ascontiguousarray(wv[:, ms]).astype(ml_dtypes.bfloat16),
            "wo": np.ascontiguousarray(wo[ms, :]).astype(ml_dtypes.bfloat16),
            "norm_w": norm_w,
            "qn_w": qn_w,
            "kn_w": kn_w,
        })
    res = run_bass_kernel_spmd(nc, in_maps, core_ids=list(range(8)))
    out = np.zeros((B, S, D), dtype=np.float32)
    for c in range(8):
        out[c // 4] += res.results[c]["out"]
    return out


# revision 19
# speedup vs baseline: 92.4878x; 1.0162x over previous
"""Trainium2 Bass kernel for nn_Attention_55894704390617.

Dense transformer attention block:
  xn = LN(x) ; q,k,v = xn @ wq/wk/wv ; q,k = headLN(q),headLN(k)
  out = softmax(q k^T / sqrt(dh)) v @ wo

Sharding over 8 NeuronCores: 2 (batch) x 4 (head groups of 8 heads).
Each core computes a partial output (its head-group's contribution to
out = attn_out @ wo); the host sums the 4 partials per batch.

Per-core data flow (matmuls in bf16, fp32 PSUM accumulation):
  - host supplies x[b] natural [S,D] fp32 (LN stats) and transposed
    [D,S] pre-cast to bf16 (the projections' contraction operand)
  - LN stats per token via bn_stats on natural tiles; each projection's
    mean-correction is folded into its matmul group as a K=1
    accumulation row (mu[s] x -colsum(w)); rstd applied per-partition
    on the PSUM results; norm_w folded into the weights on ScalarE
  - K,V projected in phase A; Q projected per 512-query chunk inside
    the attention loop so its PE work fills softmax (ACT) stalls
  - scores computed transposed (scoresT[j,i]) with two heads packed
    into PE row-groups 0/64; softmax denominators come from a
    ones-column in v_ext through the attention matmul (no
    max-subtraction: scores are O(5), fp32-safe)
  - attn output normalized from an SBUF copy (frees the PSUM
    accumulator early); denominator broadcast via a K=1 ones-matmul
  - out-proj consumes avT directly as the stationary operand
"""

import numpy as np

S = 2048          # sequence length
D = 2048          # model dim
H_LOC = 8         # heads per core
DH = 64           # head dim
M_LOC = H_LOC * DH  # 512 inner dim per core
N_D = D // 128    # 16 d-tiles
N_S = S // 128    # 16 s-tiles
N_SC = S // 512   # 4 512-chunks
N_MT = M_LOC // 128  # 4 m-tiles per core
EPS = 1e-5

_COMPILED = {}


def _build():
    from concourse._compat import axon_active
    axon_active()
    import concourse.bacc as bacc
    import concourse.mybir as mybir
    import concourse.tile as tile
    from concourse.bass import AP
    from concourse.masks import make_identity
    from contextlib import ExitStack

    F32 = mybir.dt.float32
    BF16 = mybir.dt.bfloat16
    AF = mybir.ActivationFunctionType
    OP = mybir.AluOpType

    nc = bacc.Bacc(None, target_bir_lowering=False)

    x_nat = nc.dram_tensor("x_nat", [S, D], BF16, kind="ExternalInput")
    x_tr = nc.dram_tensor("x_tr", [D, S], BF16, kind="ExternalInput")
    wq = nc.dram_tensor("wq", [D, M_LOC], BF16, kind="ExternalInput")
    wk = nc.dram_tensor("wk", [D, M_LOC], BF16, kind="ExternalInput")
    wv = nc.dram_tensor("wv", [D, M_LOC], BF16, kind="ExternalInput")
    wo = nc.dram_tensor("wo", [M_LOC, D], BF16, kind="ExternalInput")
    norm_w = nc.dram_tensor("norm_w", [D], F32, kind="ExternalInput")
    qn_w = nc.dram_tensor("qn_w", [DH], F32, kind="ExternalInput")
    kn_w = nc.dram_tensor("kn_w", [DH], F32, kind="ExternalInput")
    out = nc.dram_tensor("out", [S, D], F32, kind="ExternalOutput")

    with tile.TileContext(nc) as tc:
        es = ExitStack()
        # ---- pools alive for the whole kernel ----
        consts = es.enter_context(tc.tile_pool(name="consts", bufs=1))
        dram = es.enter_context(tc.tile_pool(name="dram", bufs=1, space="DRAM"))
        xt_pool = es.enter_context(tc.tile_pool(name="xt", bufs=1))
        wgq_pool = es.enter_context(tc.tile_pool(name="wgq", bufs=1))
        kT_pool = es.enter_context(tc.tile_pool(name="kT", bufs=1))
        vext_pool = es.enter_context(tc.tile_pool(name="vext", bufs=1))

        ident = consts.tile([128, 128], BF16, name="ident")
        make_identity(nc, ident)
        ones_all = consts.tile([128, 64], BF16, name="ones_all")
        nc.vector.memset(ones_all, 1.0)
        ones_col = consts.tile([128, 1], BF16, name="ones_col")
        nc.vector.memset(ones_col, 1.0)
        eps_t = consts.tile([128, 1], F32, name="eps_t")
        nc.vector.memset(eps_t, EPS)

        qn_rep = consts.tile([128, H_LOC, DH], F32, name="qn_rep")
        kn_rep = consts.tile([128, H_LOC, DH], F32, name="kn_rep")
        g_col = consts.tile([128, N_D], F32, name="g_col")

        # per-s-tile stat tiles (separate tiles keep deps per-tile)
        mu_col = [consts.tile([128, 1], F32, name=f"mu_col{t}") for t in range(N_S)]
        rstd_col = [consts.tile([128, 1], F32, name=f"rstd_col{t}") for t in range(N_S)]
        mu_bf = [consts.tile([1, 128], BF16, name=f"mu_bf{t}") for t in range(N_S)]
        negc_bf = {w: consts.tile([1, M_LOC], BF16, name=f"negc_{w}")
                   for w in ("q", "k", "v")}

        dscr = dram.tile([S], F32)

        wg_q = [wgq_pool.tile([128, M_LOC], BF16, name=f"wg_q{t}") for t in range(N_D)]
        xt = [xt_pool.tile([128, S], BF16, name=f"xt{t}") for t in range(N_D)]
        kT = [kT_pool.tile([128, S], BF16, name=f"kT{mt}") for mt in range(N_MT)]
        v_ext = [vext_pool.tile([128, H_LOC, DH + 1], BF16, name=f"vext{st}")
                 for st in range(N_S)]

        # ============ phase A: prep + stats + K,V projections ============
        with ExitStack() as ph:
            wg_pool = ph.enter_context(tc.tile_pool(name="wg", bufs=1))
            stage = ph.enter_context(tc.tile_pool(name="stage", bufs=3))
            wstage = ph.enter_context(tc.tile_pool(name="wstage", bufs=3))
            scrA = ph.enter_context(tc.tile_pool(name="scrA", bufs=3))
            ps_mm = ph.enter_context(tc.tile_pool(name="ps_mm", bufs=4, space="PSUM"))
            ps_tp = ph.enter_context(tc.tile_pool(name="ps_tp", bufs=2, space="PSUM"))
            ps_row = ph.enter_context(tc.tile_pool(name="ps_row", bufs=2, space="PSUM"))

            wg = {"q": wg_q}
            for wname in ("k", "v"):
                wg[wname] = [wg_pool.tile([128, M_LOC], BF16, name=f"wg_{wname}{t}")
                             for t in range(N_D)]
            wdrams = {"q": wq, "k": wk, "v": wv}

            def emit_stats(st):
                xst = stage.tile([128, S], BF16, tag="xst")
                nc.sync.dma_start(out=xst, in_=x_nat[st * 128:(st + 1) * 128, :])
                xg = xst.rearrange("p (n f) -> p n f", f=512)
                bn = scrA.tile([128, 4, 6], F32, tag="bn")
                for sg in range(4):
                    nc.vector.bn_stats(out=bn[:, sg, :], in_=xg[:, sg, :])
                mv = scrA.tile([128, 2], F32, tag="mv")
                nc.vector.bn_aggr(out=mv, in_=bn)
                nc.vector.tensor_copy(mu_col[st], mv[:, 0:1])
                nc.scalar.activation(out=rstd_col[st], in_=mv[:, 1:2],
                                     func=AF.Sqrt, bias=eps_t, scale=1.0)
                nc.vector.reciprocal(rstd_col[st], rstd_col[st])
                nc.sync.dma_start(
                    out=dscr[st * 128:(st + 1) * 128].rearrange("(p one) -> p one", one=1),
                    in_=mu_col[st])
                mur = scrA.tile([1, 128], F32, tag="mur")
                nc.sync.dma_start(
                    out=mur,
                    in_=dscr[st * 128:(st + 1) * 128].rearrange("(one s) -> one s", one=1))
                nc.vector.tensor_copy(mu_bf[st], mur)

            # interleave K-weight folds with the x^T loads so the first
            # K-projection can start as early as the DMAs allow
            cwk_ps = ps_row.tile([1, M_LOC], F32, tag="cw", name="cw_k")
            for t in range(N_D):
                nc.sync.dma_start(
                    out=g_col[:, t:t + 1],
                    in_=norm_w[t * 128:(t + 1) * 128].rearrange("(p one) -> p one",
                                                               one=1))
                wst = wstage.tile([128, M_LOC], BF16, tag="wst")
                nc.sync.dma_start(out=wst, in_=wk[t * 128:(t + 1) * 128, :])
                nc.scalar.mul(out=wg["k"][t], in_=wst, mul=g_col[:, t:t + 1])
                nc.tensor.matmul(cwk_ps[:, :], ones_col[:, :], wg["k"][t][:, :],
                                 start=(t == 0), stop=(t == N_D - 1))
                nc.sync.dma_start(out=xt[t], in_=x_tr[t * 128:(t + 1) * 128, :])
                if t == 0:
                    emit_stats(0)
            nc.vector.tensor_scalar_mul(out=negc_bf["k"], in0=cwk_ps, scalar1=-1.0)

            for rep, wten in ((qn_rep, qn_w), (kn_rep, kn_w)):
                bsrc = AP(tensor=wten[:].tensor, offset=wten[:].offset,
                          ap=[[0, 128], [0, H_LOC], [1, DH]])
                nc.sync.dma_start(out=rep, in_=bsrc)

            def emit_folds(wname):
                # stage weight, fold norm_w on ScalarE, colsum on PE
                cw_ps = ps_row.tile([1, M_LOC], F32, tag="cw", name=f"cw_{wname}")
                for t in range(N_D):
                    wst = wstage.tile([128, M_LOC], BF16, tag="wst")
                    nc.sync.dma_start(out=wst,
                                      in_=wdrams[wname][t * 128:(t + 1) * 128, :])
                    nc.scalar.mul(out=wg[wname][t], in_=wst, mul=g_col[:, t:t + 1])
                    nc.tensor.matmul(cw_ps[:, :], ones_col[:, :], wg[wname][t][:, :],
                                     start=(t == 0), stop=(t == N_D - 1))
                nc.vector.tensor_scalar_mul(out=negc_bf[wname], in0=cw_ps,
                                            scalar1=-1.0)

            emit_folds("v")

            # LN stats for the remaining tiles
            for st in range(1, N_S):
                emit_stats(st)

            # K and V projections
            for st in range(N_S):
                sl = slice(st * 128, (st + 1) * 128)
                for wname in ("k", "v"):
                    p = ps_mm.tile([128, M_LOC], F32, tag="mm")
                    for t in range(N_D):
                        nc.tensor.matmul(p[:, :], xt[t][:, sl], wg[wname][t][:, :],
                                         start=(t == 0), stop=False)
                    nc.tensor.matmul(p[:, :], mu_bf[st][:, :], negc_bf[wname][:, :],
                                     start=False, stop=True)
                    if wname == "v":
                        nc.scalar.mul(
                            out=v_ext[st][:, :, 0:DH],
                            in_=p.rearrange("p (h d) -> p h d", d=DH),
                            mul=rstd_col[st])
                        nc.vector.memset(v_ext[st][:, :, DH:DH + 1], 1.0)
                    else:
                        nat = scrA.tile([128, M_LOC], F32, tag="nat")
                        nc.scalar.mul(out=nat, in_=p, mul=rstd_col[st])
                        natg = nat.rearrange("p (h d) -> p h d", d=DH)
                        bn8 = scrA.tile([128, H_LOC, 6], F32, tag="bn8")
                        mv8 = scrA.tile([128, H_LOC, 2], F32, tag="mv8")
                        for h in range(H_LOC):
                            nc.vector.bn_stats(out=bn8[:, h, :], in_=natg[:, h, :])
                            nc.vector.bn_aggr(out=mv8[:, h, :], in_=bn8[:, h, :])
                        rstd8 = scrA.tile([128, H_LOC], F32, tag="rstd8")
                        nc.scalar.activation(out=rstd8, in_=mv8[:, :, 1], func=AF.Sqrt,
                                             bias=eps_t, scale=1.0)
                        nc.vector.reciprocal(rstd8, rstd8)
                        for h in range(H_LOC):
                            nc.vector.tensor_scalar(
                                out=natg[:, h, :], in0=natg[:, h, :],
                                scalar1=mv8[:, h, 0:1], scalar2=rstd8[:, h:h + 1],
                                op0=OP.subtract, op1=OP.mult)
                        lnb = scrA.tile([128, M_LOC], BF16, tag="lnb")
                        nc.vector.tensor_mul(out=lnb, in0=nat,
                                             in1=kn_rep.rearrange("p h d -> p (h d)"))
                        for mt in range(N_MT):
                            tp = ps_tp.tile([128, 128], BF16, tag="tp")
                            nc.tensor.transpose(
                                tp[:, :], lnb[:, mt * 128:(mt + 1) * 128], ident[:, :])
                            nc.vector.tensor_copy(kT[mt][:, sl], tp[:, :])

            # Q weight fold last: its DMAs land while the projections run
            emit_folds("q")

        # ============ phase B: Q proj + attention + out-proj ============
        with ExitStack() as ph:
            wo_pool = ph.enter_context(tc.tile_pool(name="wop", bufs=1))
            stage2 = ph.enter_context(tc.tile_pool(name="stage2", bufs=2))
            avT_pool = ph.enter_context(tc.tile_pool(name="avT", bufs=2))
            qT_pool = ph.enter_context(tc.tile_pool(name="qT", bufs=2))
            scrB = ph.enter_context(tc.tile_pool(name="scrB", bufs=2))
            attn_pool = ph.enter_context(tc.tile_pool(name="attn", bufs=8))
            dn_pool = ph.enter_context(tc.tile_pool(name="dn", bufs=3))
            avsh_pool = ph.enter_context(tc.tile_pool(name="avsh", bufs=2))
            osb_pool = ph.enter_context(tc.tile_pool(name="osb", bufs=3))
            # PSUM: sc x2 + av x2 + qp x2 + tpb x2 = 8 banks. qp also takes
            # the out-proj accumulators; tpb also takes the denominator
            # broadcasts.
            ps_sc = ph.enter_context(tc.tile_pool(name="ps_sc", bufs=2, space="PSUM"))
            ps_av = ph.enter_context(tc.tile_pool(name="ps_av", bufs=2, space="PSUM"))
            ps_qp = ph.enter_context(tc.tile_pool(name="ps_qp", bufs=2, space="PSUM"))
            ps_tpb = ph.enter_context(tc.tile_pool(name="ps_tpb", bufs=2, space="PSUM"))

            wo_bf = [wo_pool.tile([128, D], BF16, name=f"wo{mt}") for mt in range(N_MT)]
            for mt in range(N_MT):
                wst = stage2.tile([128, D], BF16, tag="wst2")
                nc.sync.dma_start(out=wst, in_=wo[mt * 128:(mt + 1) * 128, :])
                nc.vector.tensor_copy(wo_bf[mt], wst)

            def alloc_qT(ic):
                return [qT_pool.tile([128, 512], BF16, tag=f"qT{mt}",
                                     name=f"qT{mt}_{ic}") for mt in range(N_MT)]

            def emit_qproj_tail(st, p, qT):
                # everything after the matmul accumulation for one Q s-tile
                ssl = slice((st % 4) * 128, (st % 4 + 1) * 128)
                nc.tensor.matmul(p[:, :], mu_bf[st][:, :], negc_bf["q"][:, :],
                                 start=False, stop=True)
                nat = scrB.tile([128, M_LOC], F32, tag="nat")
                nc.vector.tensor_scalar_mul(out=nat, in0=p, scalar1=rstd_col[st])
                natg = nat.rearrange("p (h d) -> p h d", d=DH)
                bn8 = scrB.tile([128, H_LOC, 6], F32, tag="bn8")
                mv8 = scrB.tile([128, H_LOC, 2], F32, tag="mv8")
                for h in range(H_LOC):
                    nc.vector.bn_stats(out=bn8[:, h, :], in_=natg[:, h, :])
                    nc.vector.bn_aggr(out=mv8[:, h, :], in_=bn8[:, h, :])
                rstd8 = scrB.tile([128, H_LOC], F32, tag="rstd8")
                nc.scalar.activation(out=rstd8, in_=mv8[:, :, 1], func=AF.Sqrt,
                                     bias=eps_t, scale=1.0)
                nc.vector.reciprocal(rstd8, rstd8)
                for h in range(H_LOC):
                    nc.vector.tensor_scalar(
                        out=natg[:, h, :], in0=natg[:, h, :],
                        scalar1=mv8[:, h, 0:1], scalar2=rstd8[:, h:h + 1],
                        op0=OP.subtract, op1=OP.mult)
                lnb = scrB.tile([128, M_LOC], BF16, tag="lnb")
                nc.vector.tensor_mul(out=lnb, in0=nat,
                                     in1=qn_rep.rearrange("p h d -> p (h d)"))
                for mt in range(N_MT):
                    tp = ps_tpb.tile([128, 128], BF16, tag="tpb",
                                     name=f"tp{st}_{mt}")
                    nc.tensor.transpose(
                        tp[:, :], lnb[:, mt * 128:(mt + 1) * 128], ident[:, :])
                    nc.vector.tensor_copy(qT[mt][:, ssl], tp[:, :])

            def emit_qproj_full(ic, qT):
                for st in range(4 * ic, 4 * ic + 4):
                    sl = slice(st * 128, (st + 1) * 128)
                    p = ps_qp.tile([128, M_LOC], F32, tag="qp", name=f"qp{st}")
                    for t in range(N_D):
                        nc.tensor.matmul(p[:, :], xt[t][:, sl], wg_q[t][:, :],
                                         start=(t == 0), stop=False)
                    emit_qproj_tail(st, p, qT)

            def emit_oproj_group(avT_prev, ic_prev, st, do):
                sl = slice(st * 128, (st + 1) * 128)
                lsl = slice((st % 4) * 128, (st % 4 + 1) * 128)
                op = ps_qp.tile([128, 512], F32, tag="qp", name=f"op{st}_{do}")
                for mt in range(N_MT):
                    nc.tensor.matmul(op[:, :], avT_prev[mt][:, lsl],
                                     wo_bf[mt][:, do * 512:(do + 1) * 512],
                                     start=(mt == 0), stop=(mt == N_MT - 1))
                ot = osb_pool.tile([128, 512], F32, tag="ot")
                nc.vector.tensor_copy(ot, op)
                nc.sync.dma_start(out=out[sl, do * 512:(do + 1) * 512], in_=ot)

            qT_cur = alloc_qT(0)
            emit_qproj_full(0, qT_cur)
            avT_prev = None
            for ic in range(N_SC):
                qT = qT_cur
                qT_next = alloc_qT(ic + 1) if ic + 1 < N_SC else None
                avT = [avT_pool.tile([128, 512], BF16, tag=f"avT{mt}",
                                     name=f"avT{mt}_{ic}") for mt in range(N_MT)]
                for hp in range(N_MT):
                    av_ps = [ps_av.tile([128, 512], F32, tag="av", name=f"av{_h}")
                             for _h in range(2)]
                    # this hp's interleaved extra PE work:
                    # - s-tile hp of the next chunk's Q projection
                    # - s-tile hp of the previous chunk's out-projection
                    qst = 4 * (ic + 1) + hp if qT_next is not None else None
                    qp = None
                    if qst is not None:
                        qp = ps_qp.tile([128, M_LOC], F32, tag="qp", name=f"qp{qst}")
                    for jt in range(N_S):
                        jsl = slice(jt * 128, (jt + 1) * 128)
                        at = []
                        for hs in range(2):
                            psl = slice(hs * 64, (hs + 1) * 64)
                            sc = ps_sc.tile([128, 512], F32, tag="sc",
                                            name=f"sc{jt}_{hs}")
                            nc.tensor.matmul(sc[:, :], kT[hp][psl, jsl],
                                             qT[hp][psl, :], start=True, stop=True)
                            a = attn_pool.tile([128, 512], BF16, tag="attn")
                            nc.scalar.activation(out=a, in_=sc, func=AF.Exp,
                                                 scale=0.125)
                            at.append(a)
                        for hs in range(2):
                            nc.tensor.matmul(av_ps[hs][0:DH + 1, :],
                                             v_ext[jt][:, 2 * hp + hs, :],
                                             at[hs][:, :],
                                             start=(jt == 0), stop=(jt == N_S - 1))
                        if qp is not None:
                            qsl = slice(qst * 128, (qst + 1) * 128)
                            nc.tensor.matmul(qp[:, :], xt[jt][:, qsl],
                                             wg_q[jt][:, :],
                                             start=(jt == 0), stop=False)
                        if avT_prev is not None and jt % 4 == 0:
                            emit_oproj_group(avT_prev, ic - 1,
                                             4 * (ic - 1) + hp, jt // 4)
                    if qp is not None:
                        emit_qproj_tail(qst, qp, qT_next)
                    for hs in range(2):
                        # free the PSUM accumulator with one copy, then
                        # normalize out of SBUF
                        av_sb = dn_pool.tile([65, 512], F32, tag="avsb")
                        nc.vector.tensor_copy(av_sb, av_ps[hs][0:DH + 1, :])
                        nc.vector.reciprocal(av_sb[DH:DH + 1, :], av_sb[DH:DH + 1, :])
                        dn = dn_pool.tile([65, 512], BF16, tag="dn")
                        nc.vector.tensor_copy(dn[DH:DH + 1, :], av_sb[DH:DH + 1, :])
                        bc = ps_tpb.tile([64, 512], F32, tag="tpb", name=f"bc{hs}")
                        nc.tensor.matmul(bc[:, :], ones_all[DH:DH + 1, 0:DH],
                                         dn[DH:DH + 1, :], start=True, stop=True)
                        bc_sb = dn_pool.tile([64, 512], F32, tag="bcsb")
                        nc.vector.tensor_copy(bc_sb, bc[:, :])
                        if hs == 0:
                            nc.vector.tensor_mul(out=avT[hp][0:DH, :],
                                                 in0=av_sb[0:DH, :], in1=bc_sb)
                        else:
                            sh = avsh_pool.tile([64, 512], BF16, tag="avsh")
                            nc.vector.tensor_mul(out=sh, in0=av_sb[0:DH, :],
                                                 in1=bc_sb)
                            nc.sync.dma_start(out=avT[hp][DH:128, :], in_=sh)
                avT_prev = avT
                qT_cur = qT_next

            # tail: out-projection of the last chunk
            for st in range(4 * (N_SC - 1), 4 * N_SC):
                for do in range(N_SC):
                    emit_oproj_group(avT_prev, N_SC - 1, st, do)
        es.close()

    nc.compile()
    return nc


def _get_nc():
    if "nc" not in _COMPILED:
        _COMPILED["nc"] = _build()
    return _COMPILED["nc"]


def kernel(x, norm_w, wq, wk, wv, qn_w, kn_w, wo):
    import ml_dtypes
    from concourse.bass_utils import run_bass_kernel_spmd

    x = np.asarray(x, dtype=np.float32)
    norm_w = np.asarray(norm_w, dtype=np.float32)
    wq = np.asarray(wq, dtype=np.float32)
    wk = np.asarray(wk, dtype=np.float32)
    wv = np.asarray(wv, dtype=np.float32)
    qn_w = np.asarray(qn_w, dtype=np.float32)
    kn_w = np.asarray(kn_w, dtype=np.float32)
    wo = np.asarray(wo, dtype=np.float32)
    B = x.shape[0]

    nc = _get_nc()
    in_maps = []
    for c in range(8):
        b, g = c // 4, c % 4
        ms = slice(g * M_LOC, (g + 1) * M_LOC)
        in_maps.append({
            "x_nat": np.ascontiguousarray(x[b]).astype(ml_dtypes.bfloat16),
            "x_tr": np.ascontiguousarray(x[b].T).astype(ml_dtypes.bfloat16),
            "wq": np.ascontiguousarray(wq[:, ms]).astype(ml_dtypes.bfloat16),
            "wk": np.ascontiguousarray(wk[:, ms]).astype(ml_dtypes.bfloat16),
            "wv": np.ascontiguousarray(wv[:, ms]).astype(ml_dtypes.bfloat16),
            "wo": np.ascontiguousarray(wo[ms, :]).astype(ml_dtypes.bfloat16),
            "norm_w": norm_w,
            "qn_w": qn_w,
            "kn_w": kn_w,
        })
    res = run_bass_kernel_spmd(nc, in_maps, core_ids=list(range(8)))
    out = np.zeros((B, S, D), dtype=np.float32)
    for c in range(8):
        out[c // 4] += res.results[c]["out"]
    return out


# revision 22
# speedup vs baseline: 92.5617x; 1.0008x over previous
"""Trainium2 Bass kernel for nn_Attention_55894704390617.

Dense transformer attention block:
  xn = LN(x) ; q,k,v = xn @ wq/wk/wv ; q,k = headLN(q),headLN(k)
  out = softmax(q k^T / sqrt(dh)) v @ wo

Sharding over 8 NeuronCores: 2 (batch) x 4 (head groups of 8 heads).
Each core computes a partial output (its head-group's contribution to
out = attn_out @ wo); the host sums the 4 partials per batch.

Per-core data flow (matmuls in bf16, fp32 PSUM accumulation):
  - host supplies x[b] natural [S,D] fp32 (LN stats) and transposed
    [D,S] pre-cast to bf16 (the projections' contraction operand)
  - LN stats per token via bn_stats on natural tiles; each projection's
    mean-correction is folded into its matmul group as a K=1
    accumulation row (mu[s] x -colsum(w)); rstd applied per-partition
    on the PSUM results; norm_w folded into the weights on ScalarE
  - K,V projected in phase A; Q projected per 512-query chunk inside
    the attention loop so its PE work fills softmax (ACT) stalls
  - scores computed transposed (scoresT[j,i]) with two heads packed
    into PE row-groups 0/64; softmax denominators come from a
    ones-column in v_ext through the attention matmul (no
    max-subtraction: scores are O(5), fp32-safe)
  - attn output normalized from an SBUF copy (frees the PSUM
    accumulator early); denominator broadcast via a K=1 ones-matmul
  - out-proj consumes avT directly as the stationary operand
"""

import numpy as np

S = 2048          # sequence length
D = 2048          # model dim
H_LOC = 8         # heads per core
DH = 64           # head dim
M_LOC = H_LOC * DH  # 512 inner dim per core
N_D = D // 128    # 16 d-tiles
N_S = S // 128    # 16 s-tiles
N_SC = S // 512   # 4 512-chunks
N_MT = M_LOC // 128  # 4 m-tiles per core
EPS = 1e-5

_COMPILED = {}


def _build():
    from concourse._compat import axon_active
    axon_active()
    import concourse.bacc as bacc
    import concourse.mybir as mybir
    import concourse.tile as tile
    from concourse.bass import AP
    from concourse.masks import make_identity
    from contextlib import ExitStack

    F32 = mybir.dt.float32
    BF16 = mybir.dt.bfloat16
    AF = mybir.ActivationFunctionType
    OP = mybir.AluOpType

    nc = bacc.Bacc(None, target_bir_lowering=False)

    x_nat = nc.dram_tensor("x_nat", [S, D], BF16, kind="ExternalInput")
    x_tr = nc.dram_tensor("x_tr", [D, S], BF16, kind="ExternalInput")
    wq = nc.dram_tensor("wq", [D, M_LOC], BF16, kind="ExternalInput")
    wk = nc.dram_tensor("wk", [D, M_LOC], BF16, kind="ExternalInput")
    wv = nc.dram_tensor("wv", [D, M_LOC], BF16, kind="ExternalInput")
    wo = nc.dram_tensor("wo", [M_LOC, D], BF16, kind="ExternalInput")
    norm_w = nc.dram_tensor("norm_w", [D], F32, kind="ExternalInput")
    qn_w = nc.dram_tensor("qn_w", [DH], F32, kind="ExternalInput")
    kn_w = nc.dram_tensor("kn_w", [DH], F32, kind="ExternalInput")
    out = nc.dram_tensor("out", [S, D], F32, kind="ExternalOutput")

    with tile.TileContext(nc) as tc:
        es = ExitStack()
        # ---- pools alive for the whole kernel ----
        consts = es.enter_context(tc.tile_pool(name="consts", bufs=1))
        dram = es.enter_context(tc.tile_pool(name="dram", bufs=1, space="DRAM"))
        xt_pool = es.enter_context(tc.tile_pool(name="xt", bufs=1))
        wgq_pool = es.enter_context(tc.tile_pool(name="wgq", bufs=1))
        kT_pool = es.enter_context(tc.tile_pool(name="kT", bufs=1))
        vext_pool = es.enter_context(tc.tile_pool(name="vext", bufs=1))

        ident = consts.tile([128, 128], BF16, name="ident")
        make_identity(nc, ident)
        ones_all = consts.tile([128, 64], BF16, name="ones_all")
        nc.vector.memset(ones_all, 1.0)
        ones_col = consts.tile([128, 1], BF16, name="ones_col")
        nc.vector.memset(ones_col, 1.0)
        eps_t = consts.tile([128, 1], F32, name="eps_t")
        nc.vector.memset(eps_t, EPS)

        qn_rep = consts.tile([128, H_LOC, DH], F32, name="qn_rep")
        kn_rep = consts.tile([128, H_LOC, DH], F32, name="kn_rep")
        g_col = consts.tile([128, N_D], F32, name="g_col")

        # per-s-tile stat tiles (separate tiles keep deps per-tile)
        mu_col = [consts.tile([128, 1], F32, name=f"mu_col{t}") for t in range(N_S)]
        rstd_col = [consts.tile([128, 1], F32, name=f"rstd_col{t}") for t in range(N_S)]
        mu_bf = [consts.tile([1, 128], BF16, name=f"mu_bf{t}") for t in range(N_S)]
        negc_bf = {w: consts.tile([1, M_LOC], BF16, name=f"negc_{w}")
                   for w in ("q", "k", "v")}

        dscr = dram.tile([S], F32)

        wg_q = [wgq_pool.tile([128, M_LOC], BF16, name=f"wg_q{t}") for t in range(N_D)]
        xt = [xt_pool.tile([128, S], BF16, name=f"xt{t}") for t in range(N_D)]
        kT = [kT_pool.tile([128, S], BF16, name=f"kT{mt}") for mt in range(N_MT)]
        v_ext = [vext_pool.tile([128, H_LOC, DH + 1], BF16, name=f"vext{st}")
                 for st in range(N_S)]

        # ============ phase A: prep + stats + K,V projections ============
        with ExitStack() as ph:
            wg_pool = ph.enter_context(tc.tile_pool(name="wg", bufs=1))
            stage = ph.enter_context(tc.tile_pool(name="stage", bufs=3))
            wstage = ph.enter_context(tc.tile_pool(name="wstage", bufs=3))
            scrA = ph.enter_context(tc.tile_pool(name="scrA", bufs=3))
            ps_mm = ph.enter_context(tc.tile_pool(name="ps_mm", bufs=4, space="PSUM"))
            ps_tp = ph.enter_context(tc.tile_pool(name="ps_tp", bufs=2, space="PSUM"))
            ps_row = ph.enter_context(tc.tile_pool(name="ps_row", bufs=2, space="PSUM"))

            wg = {"q": wg_q}
            for wname in ("k", "v"):
                wg[wname] = [wg_pool.tile([128, M_LOC], BF16, name=f"wg_{wname}{t}")
                             for t in range(N_D)]
            wdrams = {"q": wq, "k": wk, "v": wv}

            def emit_stats(st):
                xst = stage.tile([128, S], BF16, tag="xst")
                nc.sync.dma_start(out=xst, in_=x_nat[st * 128:(st + 1) * 128, :])
                xg = xst.rearrange("p (n f) -> p n f", f=512)
                bn = scrA.tile([128, 4, 6], F32, tag="bn")
                for sg in range(4):
                    nc.vector.bn_stats(out=bn[:, sg, :], in_=xg[:, sg, :])
                mv = scrA.tile([128, 2], F32, tag="mv")
                nc.vector.bn_aggr(out=mv, in_=bn)
                nc.vector.tensor_copy(mu_col[st], mv[:, 0:1])
                nc.scalar.activation(out=rstd_col[st], in_=mv[:, 1:2],
                                     func=AF.Sqrt, bias=eps_t, scale=1.0)
                nc.vector.reciprocal(rstd_col[st], rstd_col[st])
                nc.sync.dma_start(
                    out=dscr[st * 128:(st + 1) * 128].rearrange("(p one) -> p one", one=1),
                    in_=mu_col[st])
                mur = scrA.tile([1, 128], F32, tag="mur")
                nc.sync.dma_start(
                    out=mur,
                    in_=dscr[st * 128:(st + 1) * 128].rearrange("(one s) -> one s", one=1))
                nc.vector.tensor_copy(mu_bf[st], mur)

            # interleave K-weight folds with the x^T loads so the first
            # K-projection can start as early as the DMAs allow
            cwk_ps = ps_row.tile([1, M_LOC], F32, tag="cw", name="cw_k")
            for t in range(N_D):
                nc.sync.dma_start(
                    out=g_col[:, t:t + 1],
                    in_=norm_w[t * 128:(t + 1) * 128].rearrange("(p one) -> p one",
                                                               one=1))
                wst = wstage.tile([128, M_LOC], BF16, tag="wst")
                nc.sync.dma_start(out=wst, in_=wk[t * 128:(t + 1) * 128, :])
                nc.scalar.mul(out=wg["k"][t], in_=wst, mul=g_col[:, t:t + 1])
                nc.tensor.matmul(cwk_ps[:, :], ones_col[:, :], wg["k"][t][:, :],
                                 start=(t == 0), stop=(t == N_D - 1))
                nc.sync.dma_start(out=xt[t], in_=x_tr[t * 128:(t + 1) * 128, :])
                if t == 0:
                    emit_stats(0)
            nc.vector.tensor_scalar_mul(out=negc_bf["k"], in0=cwk_ps, scalar1=-1.0)

            for rep, wten in ((qn_rep, qn_w), (kn_rep, kn_w)):
                bsrc = AP(tensor=wten[:].tensor, offset=wten[:].offset,
                          ap=[[0, 128], [0, H_LOC], [1, DH]])
                nc.sync.dma_start(out=rep, in_=bsrc)

            def emit_folds(wname):
                # stage weight, fold norm_w on ScalarE, colsum on PE
                cw_ps = ps_row.tile([1, M_LOC], F32, tag="cw", name=f"cw_{wname}")
                for t in range(N_D):
                    wst = wstage.tile([128, M_LOC], BF16, tag="wst")
                    nc.sync.dma_start(out=wst,
                                      in_=wdrams[wname][t * 128:(t + 1) * 128, :])
                    nc.scalar.mul(out=wg[wname][t], in_=wst, mul=g_col[:, t:t + 1])
                    nc.tensor.matmul(cw_ps[:, :], ones_col[:, :], wg[wname][t][:, :],
                                     start=(t == 0), stop=(t == N_D - 1))
                nc.vector.tensor_scalar_mul(out=negc_bf[wname], in0=cw_ps,
                                            scalar1=-1.0)

            emit_folds("v")

            # LN stats for the remaining tiles
            for st in range(1, N_S):
                emit_stats(st)

            # K and V projections
            for st in range(N_S):
                sl = slice(st * 128, (st + 1) * 128)
                for wname in ("k", "v"):
                    p = ps_mm.tile([128, M_LOC], F32, tag="mm")
                    for t in range(N_D):
                        nc.tensor.matmul(p[:, :], xt[t][:, sl], wg[wname][t][:, :],
                                         start=(t == 0), stop=False)
                    nc.tensor.matmul(p[:, :], mu_bf[st][:, :], negc_bf[wname][:, :],
                                     start=False, stop=True)
                    if wname == "v":
                        nc.scalar.mul(
                            out=v_ext[st][:, :, 0:DH],
                            in_=p.rearrange("p (h d) -> p h d", d=DH),
                            mul=rstd_col[st])
                        nc.vector.memset(v_ext[st][:, :, DH:DH + 1], 1.0)
                    else:
                        nat = scrA.tile([128, M_LOC], F32, tag="nat")
                        nc.scalar.mul(out=nat, in_=p, mul=rstd_col[st])
                        natg = nat.rearrange("p (h d) -> p h d", d=DH)
                        bn8 = scrA.tile([128, H_LOC, 6], F32, tag="bn8")
                        mv8 = scrA.tile([128, H_LOC, 2], F32, tag="mv8")
                        for h in range(H_LOC):
                            nc.vector.bn_stats(out=bn8[:, h, :], in_=natg[:, h, :])
                            nc.vector.bn_aggr(out=mv8[:, h, :], in_=bn8[:, h, :])
                        rstd8 = scrA.tile([128, H_LOC], F32, tag="rstd8")
                        nc.scalar.activation(out=rstd8, in_=mv8[:, :, 1], func=AF.Sqrt,
                                             bias=eps_t, scale=1.0)
                        nc.vector.reciprocal(rstd8, rstd8)
                        for h in range(H_LOC):
                            nc.vector.tensor_scalar(
                                out=natg[:, h, :], in0=natg[:, h, :],
                                scalar1=mv8[:, h, 0:1], scalar2=rstd8[:, h:h + 1],
                                op0=OP.subtract, op1=OP.mult)
                        lnb = scrA.tile([128, M_LOC], BF16, tag="lnb")
                        nc.vector.tensor_mul(out=lnb, in0=nat,
                                             in1=kn_rep.rearrange("p h d -> p (h d)"))
                        for mt in range(N_MT):
                            tp = ps_tp.tile([128, 128], BF16, tag="tp")
                            nc.tensor.transpose(
                                tp[:, :], lnb[:, mt * 128:(mt + 1) * 128], ident[:, :])
                            nc.vector.tensor_copy(kT[mt][:, sl], tp[:, :])

            # Q weight fold last: its DMAs land while the projections run
            emit_folds("q")

        # ============ phase B: Q proj + attention + out-proj ============
        with ExitStack() as ph:
            wo_pool = ph.enter_context(tc.tile_pool(name="wop", bufs=1))
            stage2 = ph.enter_context(tc.tile_pool(name="stage2", bufs=2))
            avT_pool = ph.enter_context(tc.tile_pool(name="avT", bufs=2))
            qT_pool = ph.enter_context(tc.tile_pool(name="qT", bufs=2))
            scrB = ph.enter_context(tc.tile_pool(name="scrB", bufs=2))
            attn_pool = ph.enter_context(tc.tile_pool(name="attn", bufs=8))
            dn_pool = ph.enter_context(tc.tile_pool(name="dn", bufs=3))
            avsh_pool = ph.enter_context(tc.tile_pool(name="avsh", bufs=2))
            osb_pool = ph.enter_context(tc.tile_pool(name="osb", bufs=3))
            # PSUM: sc x2 + av x2 + qp x2 + tpb x2 = 8 banks. qp also takes
            # the out-proj accumulators; tpb also takes the denominator
            # broadcasts.
            ps_sc = ph.enter_context(tc.tile_pool(name="ps_sc", bufs=2, space="PSUM"))
            ps_av = ph.enter_context(tc.tile_pool(name="ps_av", bufs=2, space="PSUM"))
            ps_qp = ph.enter_context(tc.tile_pool(name="ps_qp", bufs=2, space="PSUM"))
            ps_tpb = ph.enter_context(tc.tile_pool(name="ps_tpb", bufs=2, space="PSUM"))

            wo_bf = [wo_pool.tile([128, D], BF16, name=f"wo{mt}") for mt in range(N_MT)]
            for mt in range(N_MT):
                wst = stage2.tile([128, D], BF16, tag="wst2")
                nc.sync.dma_start(out=wst, in_=wo[mt * 128:(mt + 1) * 128, :])
                nc.vector.tensor_copy(wo_bf[mt], wst)

            def alloc_qT(ic):
                return [qT_pool.tile([128, 512], BF16, tag=f"qT{mt}",
                                     name=f"qT{mt}_{ic}") for mt in range(N_MT)]

            def emit_qproj_tail(st, p, qT):
                # everything after the matmul accumulation for one Q s-tile
                ssl = slice((st % 4) * 128, (st % 4 + 1) * 128)
                nc.tensor.matmul(p[:, :], mu_bf[st][:, :], negc_bf["q"][:, :],
                                 start=False, stop=True)
                nat = scrB.tile([128, M_LOC], F32, tag="nat")
                nc.vector.tensor_scalar_mul(out=nat, in0=p, scalar1=rstd_col[st])
                natg = nat.rearrange("p (h d) -> p h d", d=DH)
                bn8 = scrB.tile([128, H_LOC, 6], F32, tag="bn8")
                mv8 = scrB.tile([128, H_LOC, 2], F32, tag="mv8")
                for h in range(H_LOC):
                    nc.vector.bn_stats(out=bn8[:, h, :], in_=natg[:, h, :])
                    nc.vector.bn_aggr(out=mv8[:, h, :], in_=bn8[:, h, :])
                rstd8 = scrB.tile([128, H_LOC], F32, tag="rstd8")
                nc.scalar.activation(out=rstd8, in_=mv8[:, :, 1], func=AF.Sqrt,
                                     bias=eps_t, scale=1.0)
                nc.vector.reciprocal(rstd8, rstd8)
                for h in range(H_LOC):
                    nc.vector.tensor_scalar(
                        out=natg[:, h, :], in0=natg[:, h, :],
                        scalar1=mv8[:, h, 0:1], scalar2=rstd8[:, h:h + 1],
                        op0=OP.subtract, op1=OP.mult)
                lnb = scrB.tile([128, M_LOC], BF16, tag="lnb")
                nc.vector.tensor_mul(out=lnb, in0=nat,
                                     in1=qn_rep.rearrange("p h d -> p (h d)"))
                for mt in range(N_MT):
                    tp = ps_tpb.tile([128, 128], BF16, tag="tpb",
                                     name=f"tp{st}_{mt}")
                    nc.tensor.transpose(
                        tp[:, :], lnb[:, mt * 128:(mt + 1) * 128], ident[:, :])
                    nc.vector.tensor_copy(qT[mt][:, ssl], tp[:, :])

            def emit_qproj_full(ic, qT):
                for st in range(4 * ic, 4 * ic + 4):
                    sl = slice(st * 128, (st + 1) * 128)
                    p = ps_qp.tile([128, M_LOC], F32, tag="qp", name=f"qp{st}")
                    for t in range(N_D):
                        nc.tensor.matmul(p[:, :], xt[t][:, sl], wg_q[t][:, :],
                                         start=(t == 0), stop=False)
                    emit_qproj_tail(st, p, qT)

            def emit_oproj_group(avT_prev, ic_prev, st, do):
                sl = slice(st * 128, (st + 1) * 128)
                lsl = slice((st % 4) * 128, (st % 4 + 1) * 128)
                op = ps_qp.tile([128, 512], F32, tag="qp", name=f"op{st}_{do}")
                for mt in range(N_MT):
                    nc.tensor.matmul(op[:, :], avT_prev[mt][:, lsl],
                                     wo_bf[mt][:, do * 512:(do + 1) * 512],
                                     start=(mt == 0), stop=(mt == N_MT - 1))
                ot = osb_pool.tile([128, 512], F32, tag="ot")
                nc.vector.tensor_copy(ot, op)
                nc.sync.dma_start(out=out[sl, do * 512:(do + 1) * 512], in_=ot)

            qT_cur = alloc_qT(0)
            emit_qproj_full(0, qT_cur)
            avT_prev = None
            for ic in range(N_SC):
                qT = qT_cur
                qT_next = alloc_qT(ic + 1) if ic + 1 < N_SC else None
                avT = [avT_pool.tile([128, 512], BF16, tag=f"avT{mt}",
                                     name=f"avT{mt}_{ic}") for mt in range(N_MT)]
                for hp in range(N_MT):
                    av_ps = [ps_av.tile([128, 512], F32, tag="av", name=f"av{_h}")
                             for _h in range(2)]
                    # this hp's interleaved extra PE work:
                    # - s-tile hp of the next chunk's Q projection
                    # - s-tile hp of the previous chunk's out-projection
                    qst = 4 * (ic + 1) + hp if qT_next is not None else None
                    qp = None
                    if qst is not None:
                        qp = ps_qp.tile([128, M_LOC], F32, tag="qp", name=f"qp{qst}")
                    for jt in range(N_S):
                        jsl = slice(jt * 128, (jt + 1) * 128)
                        at = []
                        for hs in range(2):
                            psl = slice(hs * 64, (hs + 1) * 64)
                            sc = ps_sc.tile([128, 512], F32, tag="sc",
                                            name=f"sc{jt}_{hs}")
                            nc.tensor.matmul(sc[:, :], kT[hp][psl, jsl],
                                             qT[hp][psl, :], start=True, stop=True)
                            a = attn_pool.tile([128, 512], BF16, tag="attn")
                            nc.scalar.activation(out=a, in_=sc, func=AF.Exp,
                                                 scale=0.125)
                            at.append(a)
                        for hs in range(2):
                            nc.tensor.matmul(av_ps[hs][0:DH + 1, :],
                                             v_ext[jt][:, 2 * hp + hs, :],
                                             at[hs][:, :],
                                             start=(jt == 0), stop=(jt == N_S - 1))
                        if qp is not None:
                            qsl = slice(qst * 128, (qst + 1) * 128)
                            nc.tensor.matmul(qp[:, :], xt[jt][:, qsl],
                                             wg_q[jt][:, :],
                                             start=(jt == 0), stop=False)
                        if avT_prev is not None and jt % 4 == 2:
                            emit_oproj_group(avT_prev, ic - 1,
                                             4 * (ic - 1) + hp, jt // 4)
                    if qp is not None:
                        emit_qproj_tail(qst, qp, qT_next)
                    for hs in range(2):
                        # free the PSUM accumulator with one copy, then
                        # normalize out of SBUF
                        av_sb = dn_pool.tile([65, 512], F32, tag="avsb")
                        nc.vector.tensor_copy(av_sb, av_ps[hs][0:DH + 1, :])
                        nc.vector.reciprocal(av_sb[DH:DH + 1, :], av_sb[DH:DH + 1, :])
                        dn = dn_pool.tile([65, 512], BF16, tag="dn")
                        nc.vector.tensor_copy(dn[DH:DH + 1, :], av_sb[DH:DH + 1, :])
                        bc = ps_tpb.tile([64, 512], F32, tag="tpb", name=f"bc{hs}")
                        nc.tensor.matmul(bc[:, :], ones_all[DH:DH + 1, 0:DH],
                                         dn[DH:DH + 1, :], start=True, stop=True)
                        bc_sb = dn_pool.tile([64, 512], F32, tag="bcsb")
                        nc.vector.tensor_copy(bc_sb, bc[:, :])
                        if hs == 0:
                            nc.vector.tensor_mul(out=avT[hp][0:DH, :],
                                                 in0=av_sb[0:DH, :], in1=bc_sb)
                        else:
                            sh = avsh_pool.tile([64, 512], BF16, tag="avsh")
                            nc.vector.tensor_mul(out=sh, in0=av_sb[0:DH, :],
                                                 in1=bc_sb)
                            nc.sync.dma_start(out=avT[hp][DH:128, :], in_=sh)
                avT_prev = avT
                qT_cur = qT_next

            # tail: out-projection of the last chunk
            for st in range(4 * (N_SC - 1), 4 * N_SC):
                for do in range(N_SC):
                    emit_oproj_group(avT_prev, N_SC - 1, st, do)
        es.close()

    nc.compile()
    return nc


def _get_nc():
    if "nc" not in _COMPILED:
        _COMPILED["nc"] = _build()
    return _COMPILED["nc"]


def kernel(x, norm_w, wq, wk, wv, qn_w, kn_w, wo):
    import ml_dtypes
    from concourse.bass_utils import run_bass_kernel_spmd

    x = np.asarray(x, dtype=np.float32)
    norm_w = np.asarray(norm_w, dtype=np.float32)
    wq = np.asarray(wq, dtype=np.float32)
    wk = np.asarray(wk, dtype=np.float32)
    wv = np.asarray(wv, dtype=np.float32)
    qn_w = np.asarray(qn_w, dtype=np.float32)
    kn_w = np.asarray(kn_w, dtype=np.float32)
    wo = np.asarray(wo, dtype=np.float32)
    B = x.shape[0]

    nc = _get_nc()
    in_maps = []
    for c in range(8):
        b, g = c // 4, c % 4
        ms = slice(g * M_LOC, (g + 1) * M_LOC)
        in_maps.append({
            "x_nat": np.ascontiguousarray(x[b]).astype(ml_dtypes.bfloat16),
            "x_tr": np.ascontiguousarray(x[b].T).astype(ml_dtypes.bfloat16),
            "wq": np.ascontiguousarray(wq[:, ms]).astype(ml_dtypes.bfloat16),
            "wk": np.ascontiguousarray(wk[:, ms]).astype(ml_dtypes.bfloat16),
            "wv": np.ascontiguousarray(wv[:, ms]).astype(ml_dtypes.bfloat16),
            "wo": np.ascontiguousarray(wo[ms, :]).astype(ml_dtypes.bfloat16),
            "norm_w": norm_w,
            "qn_w": qn_w,
            "kn_w": kn_w,
        })
    res = run_bass_kernel_spmd(nc, in_maps, core_ids=list(range(8)))
    out = np.zeros((B, S, D), dtype=np.float32)
    for c in range(8):
        out[c // 4] += res.results[c]["out"]
    return out
